# revision 10
# baseline (speedup 1.0000x reference)
"""WaveNet-style gated dilated conv layer on 8 Trainium2 NeuronCores.

Strategy: data-parallel over batch (B=8 -> 1 batch element per core).
Per core (batch b):
  z_tanh = sum_k Wc_tanh[k] @ x[:, t-d*(2-k)] + Wcond_tanh @ cond + bias
  z_sig  = likewise for the second half of the 2R conv channels
  h      = tanh(z_tanh) * sigmoid(z_sig)
  out    = W_out @ h, skip = W_skip @ h  (1x1 convs)
All matmuls run in bf16 with fp32 PSUM accumulation.  x and cond are cast
to bf16 on host to halve HBM->SBUF traffic; x is also causal-padded on
host so no on-chip memset is needed.

HBM-traffic/trigger layout (all per core):
 - all weights are packed on host into ONE [128, 1280] bf16 tensor
   (6 conv-tap blocks | 2 cond blocks (rows 0:80) | out | skip), so the
   constant load is a single DMA trigger.
 - out and skip are written bf16 into ONE [128, 2T] DRAM tensor,
   chunk-interleaved ([out_chunk | skip_chunk] per chunk), so each chunk
   flush is a single DMA trigger and output traffic is halved vs fp32.
   Host de-interleaves and casts back to fp32.
 - each engine's dma_start goes to that engine's own DGE queue; one
   queue for everything is itself a bottleneck (~65-79us busy at the
   observed ~240 GB/s) and output flushes starve the input loads queued
   behind them near the tail.  Inputs (x, cond) ride the sync HWDGE
   queue, the weight load rides the scalar HWDGE queue (so it overlaps
   the chunk-0 input load at startup), and output flushes ride the
   otherwise-idle gpsimd SWDGE queue where their triggers can't punch
   holes in the activation stream.

TRN2 matmul instructions only have room for a single semaphore wait, so
the kernel is structured so no matmul ever needs two: input DMAs are
"observed" by the PE via standalone ldweights instructions before the
first matmul that would otherwise combine a DMA wait with a PSUM WAR
wait.
"""

import sys

for _p in ("/opt/trn_rl_repo",):
    if _p not in sys.path:
        sys.path.append(_p)

from contextlib import ExitStack

import ml_dtypes
import numpy as np

import concourse.bacc as bacc
import concourse.bass as bass
import concourse.tile as tile
from concourse import mybir
from concourse.bass_utils import run_bass_kernel_spmd

B, CIN, T = 8, 128, 16384
R, S, CC, KW = 128, 128, 80, 3
NT = 512           # time-tile width (one PSUM bank of fp32)
N_CORES = 8

BF16 = mybir.dt.bfloat16
FP32 = mybir.dt.float32
AF = mybir.ActivationFunctionType

# packed weight column offsets: 6 conv taps, 2 cond halves, out, skip
WC_TAN = 0              # 3 blocks of 128 (k=0,1,2), tanh half
WC_SIG = 3 * R          # 3 blocks of 128, sigmoid half
WCOND = 6 * R           # 2 blocks of 128 (rows 0:CC valid)
WOS = 8 * R             # out block then skip block
WTS_COLS = 10 * R

_built = {}
_TRACE = False        # set True (e.g. by a test harness) to capture an NTFF profile
_last_results = None  # BassKernelResults of the most recent run


# Streaming chunk widths: ramped at the head so each chunk's input DMA
# (~240 GB/s aggregate) lands before compute catches up to it, large in
# the middle (few DMA triggers), small at the tail (fast final drain).
CHUNK_WIDTHS = [512, 1024, 1536, 2048, 2560, 2560, 2560, 1536, 1024, 512, 512]
assert sum(CHUNK_WIDTHS) == T
CHUNK_STARTS = [sum(CHUNK_WIDTHS[:i]) for i in range(len(CHUNK_WIDTHS))]
NCH = len(CHUNK_WIDTHS)
PREFETCH = 2         # chunk lookahead beyond the current group
WARMUP_MM = 7        # cold matmuls to cover the initial DMA latency


def _build(dilation: int, has_zbias: bool) -> bass.Bass:
    pad = dilation * (KW - 1)

    nc = bacc.Bacc("TRN2", target_bir_lowering=False, debug=False, num_devices=N_CORES)

    x = nc.declare_dram_parameter("x", [CIN, pad + T], BF16, isOutput=False)
    cond = nc.declare_dram_parameter("cond", [CC, T], BF16, isOutput=False)
    wts = nc.declare_dram_parameter("wts", [CIN, WTS_COLS], BF16, isOutput=False)
    if has_zbias:
        zbias = nc.declare_dram_parameter("zbias", [R, 2], FP32, isOutput=False)

    outs = nc.declare_dram_parameter("outs", [R, 2 * T], BF16, isOutput=True)

    with tile.TileContext(nc) as tc, ExitStack() as ctx:
        consts = ctx.enter_context(tc.tile_pool(name="consts", bufs=1))
        inpool = ctx.enter_context(tc.tile_pool(name="inp", bufs=PREFETCH + 2))
        hpool = ctx.enter_context(tc.tile_pool(name="h", bufs=3))
        opool = ctx.enter_context(tc.tile_pool(name="o", bufs=3))
        zpsum = ctx.enter_context(tc.tile_pool(name="zpsum", bufs=2, space="PSUM"))
        opsum = ctx.enter_context(tc.tile_pool(name="opsum", bufs=2, space="PSUM"))

        # chunk 0 is loaded before anything else: every HWDGE trigger costs
        # ~0.6us of serial sequencer time, so the first-needed data goes first
        xc_tiles = [None] * NCH
        cc_tiles = [None] * NCH

        def load_chunk(g):
            gs, gw = CHUNK_STARTS[g], CHUNK_WIDTHS[g]
            xc = inpool.tile([CIN, pad + gw], BF16, tag="xc")
            nc.sync.dma_start(xc[:], x[:, gs : gs + pad + gw])
            cc = inpool.tile([CC, gw], BF16, tag="cc")
            nc.sync.dma_start(cc[:], cond[:, gs : gs + gw])
            xc_tiles[g], cc_tiles[g] = xc, cc

        wts_sb = consts.tile([CIN, WTS_COLS], BF16)
        nc.scalar.dma_start(wts_sb[:], wts[:])
        load_chunk(0)
        if has_zbias:
            zbias_sb = consts.tile([R, 2], FP32)
            nc.scalar.dma_start(zbias_sb[:], zbias[:])
            tan_bias = zbias_sb[:, 0:1]
            sig_bias = zbias_sb[:, 1:2]
        else:
            tan_bias = 0.0
            sig_bias = 0.0
        load_chunk(1)

        # Warm-up during the input-load head: a few matmuls on uninitialized
        # SBUF kick the PE HAM toward 8/8 before real work arrives, and two
        # 1-column activations trigger the tanh/sigmoid table load (~2.7us).
        garbage = consts.tile([CIN, NT], BF16)
        act_sink = consts.tile([R, 1], FP32)
        nc.vector.memset(garbage[:], 0.0)
        nc.vector.memset(act_sink[:], 0.0)
        nc.scalar.activation(act_sink[:], act_sink[:], AF.Tanh, bias=tan_bias)
        nc.scalar.activation(act_sink[:], act_sink[:], AF.Sigmoid, bias=sig_bias)
        for _ in range(WARMUP_MM):
            wz = zpsum.tile([R, NT], FP32, tag="ztan")
            nc.tensor.matmul(wz[:], garbage[:, 0:R], garbage[:], start=True, stop=True)

        for g in range(NCH):
            gs, gw = CHUNK_STARTS[g], CHUNK_WIDTHS[g]
            for gg in range(g + 1, min(g + PREFETCH + 1, NCH)):
                if xc_tiles[gg] is None:
                    load_chunk(gg)
            xc, cc = xc_tiles[g], cc_tiles[g]
            # let PE observe the chunk DMA sems on standalone 1-column
            # ldweights (a full-width observer costs a ~430ns PE bubble at
            # every chunk boundary) so no accumulating matmul needs two waits
            nc.tensor.ldweights(xc[:, 0:1])
            nc.tensor.ldweights(cc[:, 0:1])

            stg = opool.tile([R, 2 * gw], BF16, tag="stg")
            for l0 in range(0, gw, NT):
                w = min(NT, gw - l0)
                ztan = zpsum.tile([R, w], FP32, tag="ztan")
                zsig = zpsum.tile([R, w], FP32, tag="zsig")
                for k in range(KW):
                    xs = xc[:, l0 + dilation * k : l0 + dilation * k + w]
                    nc.tensor.matmul(
                        ztan[:], wts_sb[:, WC_TAN + k * R : WC_TAN + (k + 1) * R], xs,
                        start=(k == 0), stop=False,
                    )
                nc.tensor.matmul(
                    ztan[:], wts_sb[0:CC, WCOND : WCOND + R], cc[:, l0 : l0 + w],
                    start=False, stop=True,
                )
                for k in range(KW):
                    xs = xc[:, l0 + dilation * k : l0 + dilation * k + w]
                    nc.tensor.matmul(
                        zsig[:], wts_sb[:, WC_SIG + k * R : WC_SIG + (k + 1) * R], xs,
                        start=(k == 0), stop=False,
                    )
                nc.tensor.matmul(
                    zsig[:], wts_sb[0:CC, WCOND + R : WCOND + 2 * R], cc[:, l0 : l0 + w],
                    start=False, stop=True,
                )

                th = hpool.tile([R, w], BF16, tag="th")
                nc.scalar.activation(th[:], ztan[:], AF.Tanh, bias=tan_bias)
                sg = hpool.tile([R, w], BF16, tag="sg")
                nc.scalar.activation(sg[:], zsig[:], AF.Sigmoid, bias=sig_bias)
                h = hpool.tile([R, w], BF16, tag="h")
                nc.vector.tensor_mul(h[:], th[:], sg[:])

                po = opsum.tile([R, NT], FP32, tag="po")
                nc.tensor.matmul(
                    po[:], wts_sb[:, WOS : WOS + R], h[:], start=True, stop=True
                )
                ps = opsum.tile([S, NT], FP32, tag="ps")
                nc.tensor.matmul(
                    ps[:], wts_sb[:, WOS + R : WOS + R + S], h[:], start=True, stop=True
                )
                nc.vector.tensor_copy(stg[:, l0 : l0 + w], po[:])
                nc.vector.tensor_copy(stg[:, gw + l0 : gw + l0 + w], ps[:])

            nc.gpsimd.dma_start(outs[:, 2 * gs : 2 * gs + 2 * gw], stg[:])

    nc.compile()
    return nc


def _pack_weights(w_conv, w_cond, w_out, w_skip):
    bf = ml_dtypes.bfloat16
    wts_p = np.zeros((CIN, WTS_COLS), dtype=bf)
    for k in range(KW):
        wts_p[:, WC_TAN + k * R : WC_TAN + (k + 1) * R] = w_conv[0:R, :, k].T.astype(bf)
        wts_p[:, WC_SIG + k * R : WC_SIG + (k + 1) * R] = (
            w_conv[R : 2 * R, :, k].T.astype(bf)
        )
    wts_p[0:CC, WCOND : WCOND + R] = w_cond[0:R, :, 0].T.astype(bf)
    wts_p[0:CC, WCOND + R : WCOND + 2 * R] = w_cond[R : 2 * R, :, 0].T.astype(bf)
    wts_p[:, WOS : WOS + R] = w_out[:, :, 0].T.astype(bf)
    wts_p[:, WOS + R : WOS + R + S] = w_skip[:, :, 0].T.astype(bf)
    return wts_p


def kernel(**inputs):
    x = np.asarray(inputs["x"], dtype=np.float32)
    cond = np.asarray(inputs["cond"], dtype=np.float32)
    w_conv = np.asarray(inputs["w_conv"], dtype=np.float32)
    b_conv = np.asarray(inputs["b_conv"], dtype=np.float32)
    w_cond = np.asarray(inputs["w_cond"], dtype=np.float32)
    b_cond = np.asarray(inputs["b_cond"], dtype=np.float32)
    w_out = np.asarray(inputs["w_out"], dtype=np.float32)
    b_out = np.asarray(inputs["b_out"], dtype=np.float32)
    w_skip = np.asarray(inputs["w_skip"], dtype=np.float32)
    b_skip = np.asarray(inputs["b_skip"], dtype=np.float32)
    dilation = int(np.asarray(inputs["dilation"]))
    pad = dilation * (KW - 1)

    zbias_p = np.stack(
        [b_conv[:R] + b_cond[:R], b_conv[R:] + b_cond[R:]], axis=1
    ).astype(np.float32)
    has_zbias = bool(zbias_p.any())

    key = (dilation, has_zbias)
    if key not in _built:
        _built[key] = _build(dilation, has_zbias)
    nc = _built[key]

    wts_p = _pack_weights(w_conv, w_cond, w_out, w_skip)
    bf = ml_dtypes.bfloat16
    xb = np.zeros((B, CIN, pad + T), dtype=bf)
    xb[:, :, pad:] = x.astype(bf)
    cb = np.ascontiguousarray(cond.astype(bf))

    in_maps = []
    for b in range(B):
        m = {"x": xb[b], "cond": cb[b], "wts": wts_p}
        if has_zbias:
            m["zbias"] = zbias_p
        in_maps.append(m)
    br = run_bass_kernel_spmd(nc, in_maps, list(range(N_CORES)), trace=_TRACE)
    global _last_results
    _last_results = br
    res = br.results
    output = np.empty((B, R, T), dtype=np.float32)
    skip = np.empty((B, S, T), dtype=np.float32)
    for b in range(B):
        ob = np.asarray(res[b]["outs"])
        for gs, gw in zip(CHUNK_STARTS, CHUNK_WIDTHS):
            output[b, :, gs : gs + gw] = ob[:, 2 * gs : 2 * gs + gw]
            skip[b, :, gs : gs + gw] = ob[:, 2 * gs + gw : 2 * gs + 2 * gw]
    if b_out.any():
        output = output + b_out[None, :, None]
    if b_skip.any():
        skip = skip + b_skip[None, :, None]
    return (output, skip)


# revision 14
# speedup vs baseline: 1.0464x; 1.0464x over previous
"""WaveNet-style gated dilated conv layer on 8 Trainium2 NeuronCores.

Strategy: data-parallel over batch (B=8 -> 1 batch element per core).
Per core (batch b):
  z_tanh = sum_k Wc_tanh[k] @ x[:, t-d*(2-k)] + Wcond_tanh @ cond + bias
  z_sig  = likewise for the second half of the 2R conv channels
  h      = tanh(z_tanh) * sigmoid(z_sig)
  out    = W_out @ h, skip = W_skip @ h  (1x1 convs)
All matmuls run in bf16 with fp32 PSUM accumulation.  x and cond are cast
to bf16 on host to halve HBM->SBUF traffic; x is also causal-padded on
host so no on-chip memset is needed.

HBM-traffic/trigger layout (all per core):
 - all weights are packed on host into ONE [128, 1280] bf16 tensor
   (6 conv-tap blocks | 2 cond blocks (rows 0:80) | out | skip), so the
   constant load is a single DMA trigger.
 - out and skip are written bf16 into ONE [128, 2T] DRAM tensor,
   chunk-interleaved ([out_chunk | skip_chunk] per chunk), so each chunk
   flush is a single DMA trigger and output traffic is halved vs fp32.
   Host de-interleaves and casts back to fp32.
 - each engine's dma_start goes to that engine's own DGE queue; one
   queue for everything is itself a bottleneck (~65-79us busy at the
   observed ~240 GB/s) and output flushes starve the input loads queued
   behind them near the tail.  Inputs (x, cond) ride the sync HWDGE
   queue; the weight load and output flushes ride the scalar HWDGE
   queue (gpsimd SWDGE measured slower and is avoided).

The out/skip 1x1 matmuls for tile i are issued during tile i+1's z
matmuls (one-tile software pipelining): h(i) comes out of a scalar
activation + vector multiply chain that trails the PE by most of a
tile, and the PE queue is FIFO past the ldweights window, so issuing
po/ps(i) right after z(i) stalls the PE ~0.4us per tile waiting on h.

TRN2 matmul instructions only have room for a single semaphore wait, so
the kernel is structured so no matmul ever needs two: input DMAs are
"observed" by the PE via standalone ldweights instructions before the
first matmul that would otherwise combine a DMA wait with a PSUM WAR
wait.
"""

import sys

for _p in ("/opt/trn_rl_repo",):
    if _p not in sys.path:
        sys.path.append(_p)

from contextlib import ExitStack

import ml_dtypes
import numpy as np

import concourse.bacc as bacc
import concourse.bass as bass
import concourse.tile as tile
from concourse import mybir
from concourse.bass_utils import run_bass_kernel_spmd

B, CIN, T = 8, 128, 16384
R, S, CC, KW = 128, 128, 80, 3
NT = 512           # time-tile width (one PSUM bank of fp32)
N_CORES = 8

BF16 = mybir.dt.bfloat16
FP32 = mybir.dt.float32
AF = mybir.ActivationFunctionType

# packed weight column offsets: 6 conv taps, 2 cond halves, out, skip
WC_TAN = 0              # 3 blocks of 128 (k=0,1,2), tanh half
WC_SIG = 3 * R          # 3 blocks of 128, sigmoid half
WCOND = 6 * R           # 2 blocks of 128 (rows 0:CC valid)
WOS = 8 * R             # out block then skip block
WTS_COLS = 10 * R

_built = {}
_TRACE = False        # set True (e.g. by a test harness) to capture an NTFF profile
_last_results = None  # BassKernelResults of the most recent run


# Streaming chunk widths: ramped at the head so each chunk's input DMA
# (~240 GB/s aggregate) lands before compute catches up to it, large in
# the middle (few DMA triggers), small at the tail (fast final drain).
CHUNK_WIDTHS = [512, 1024, 1536, 2048, 2560, 2560, 2560, 1536, 1024, 512, 512]
assert sum(CHUNK_WIDTHS) == T
CHUNK_STARTS = [sum(CHUNK_WIDTHS[:i]) for i in range(len(CHUNK_WIDTHS))]
NCH = len(CHUNK_WIDTHS)
PREFETCH = 2         # chunk lookahead beyond the current group
WARMUP_MM = 40       # narrow (N=128) cold matmuls covering the initial DMA
                     # latency: each retires in ~107ns so they bridge the
                     # ~4-5us until chunk-0 data lands without delaying the
                     # first real matmul, and keep the PE HAM window busy so
                     # the clock is at 8/8 when real work starts


def _build(dilation: int, has_zbias: bool) -> bass.Bass:
    pad = dilation * (KW - 1)

    nc = bacc.Bacc("TRN2", target_bir_lowering=False, debug=False, num_devices=N_CORES)

    x = nc.declare_dram_parameter("x", [CIN, pad + T], BF16, isOutput=False)
    cond = nc.declare_dram_parameter("cond", [CC, T], BF16, isOutput=False)
    wts = nc.declare_dram_parameter("wts", [CIN, WTS_COLS], BF16, isOutput=False)
    if has_zbias:
        zbias = nc.declare_dram_parameter("zbias", [R, 2], FP32, isOutput=False)

    outs = nc.declare_dram_parameter("outs", [R, 2 * T], BF16, isOutput=True)

    with tile.TileContext(nc) as tc, ExitStack() as ctx:
        consts = ctx.enter_context(tc.tile_pool(name="consts", bufs=1))
        inpool = ctx.enter_context(tc.tile_pool(name="inp", bufs=PREFETCH + 2))
        hpool = ctx.enter_context(tc.tile_pool(name="h", bufs=3))
        opool = ctx.enter_context(tc.tile_pool(name="o", bufs=3))
        zpsum = ctx.enter_context(tc.tile_pool(name="zpsum", bufs=2, space="PSUM"))
        opsum = ctx.enter_context(tc.tile_pool(name="opsum", bufs=2, space="PSUM"))

        # chunk 0 is loaded before anything else: every HWDGE trigger costs
        # ~0.6us of serial sequencer time, so the first-needed data goes first
        xc_tiles = [None] * NCH
        cc_tiles = [None] * NCH

        def load_chunk(g):
            gs, gw = CHUNK_STARTS[g], CHUNK_WIDTHS[g]
            xc = inpool.tile([CIN, pad + gw], BF16, tag="xc")
            nc.sync.dma_start(xc[:], x[:, gs : gs + pad + gw])
            cc = inpool.tile([CC, gw], BF16, tag="cc")
            nc.sync.dma_start(cc[:], cond[:, gs : gs + gw])
            xc_tiles[g], cc_tiles[g] = xc, cc

        wts_sb = consts.tile([CIN, WTS_COLS], BF16)
        nc.scalar.dma_start(wts_sb[:], wts[:])
        load_chunk(0)
        if has_zbias:
            zbias_sb = consts.tile([R, 2], FP32)
            nc.scalar.dma_start(zbias_sb[:], zbias[:])
            tan_bias = zbias_sb[:, 0:1]
            sig_bias = zbias_sb[:, 1:2]
        else:
            tan_bias = 0.0
            sig_bias = 0.0
        load_chunk(1)

        # Warm-up during the input-load head: narrow matmuls on a zeroed
        # SBUF tile keep the PE HAM busy until real data arrives, and two
        # 1-column activations trigger the tanh/sigmoid table load (~2.7us).
        garbage = consts.tile([CIN, NT], BF16)
        act_sink = consts.tile([R, 1], FP32)
        nc.vector.memset(garbage[:], 0.0)
        nc.vector.memset(act_sink[:], 0.0)
        nc.scalar.activation(act_sink[:], act_sink[:], AF.Tanh, bias=tan_bias)
        nc.scalar.activation(act_sink[:], act_sink[:], AF.Sigmoid, bias=sig_bias)
        for _ in range(WARMUP_MM):
            wz = zpsum.tile([R, NT], FP32, tag="ztan")
            nc.tensor.matmul(
                wz[:, 0:R], garbage[:, 0:R], garbage[:, 0:R], start=True, stop=True
            )

        # prev = (h, stg, gs, gw, l0) of the tile whose out/skip matmuls
        # haven't been issued yet; ready_flush = chunks whose staging tile
        # is fully written and can be DMAed out.
        prev = None
        ready_flush = []

        def emit_outskip():
            nonlocal prev
            ph, pstg, pgs, pgw, pl0 = prev
            po = opsum.tile([R, NT], FP32, tag="po")
            nc.tensor.matmul(
                po[:], wts_sb[:, WOS : WOS + R], ph[:], start=True, stop=True
            )
            ps = opsum.tile([S, NT], FP32, tag="ps")
            nc.tensor.matmul(
                ps[:], wts_sb[:, WOS + R : WOS + R + S], ph[:], start=True, stop=True
            )
            nc.vector.tensor_copy(pstg[:, pl0 : pl0 + NT], po[:])
            nc.vector.tensor_copy(pstg[:, pgw + pl0 : pgw + pl0 + NT], ps[:])
            if pl0 + NT == pgw:
                ready_flush.append((pgs, pgw, pstg))
            prev = None

        def flush_ready():
            while ready_flush:
                fgs, fgw, fstg = ready_flush.pop(0)
                nc.scalar.dma_start(outs[:, 2 * fgs : 2 * fgs + 2 * fgw], fstg[:])

        for g in range(NCH):
            gs, gw = CHUNK_STARTS[g], CHUNK_WIDTHS[g]
            for gg in range(g + 1, min(g + PREFETCH + 1, NCH)):
                if xc_tiles[gg] is None:
                    load_chunk(gg)
            xc, cc = xc_tiles[g], cc_tiles[g]
            # let PE observe the chunk DMA sems on standalone 1-column
            # ldweights (a full-width observer costs a ~430ns PE bubble at
            # every chunk boundary) so no accumulating matmul needs two waits
            nc.tensor.ldweights(xc[:, 0:1])
            nc.tensor.ldweights(cc[:, 0:1])

            stg = opool.tile([R, 2 * gw], BF16, tag="stg")
            for l0 in range(0, gw, NT):
                w = min(NT, gw - l0)
                ztan = zpsum.tile([R, w], FP32, tag="ztan")
                zsig = zpsum.tile([R, w], FP32, tag="zsig")
                for k in range(KW):
                    xs = xc[:, l0 + dilation * k : l0 + dilation * k + w]
                    nc.tensor.matmul(
                        ztan[:], wts_sb[:, WC_TAN + k * R : WC_TAN + (k + 1) * R], xs,
                        start=(k == 0), stop=False,
                    )
                nc.tensor.matmul(
                    ztan[:], wts_sb[0:CC, WCOND : WCOND + R], cc[:, l0 : l0 + w],
                    start=False, stop=True,
                )
                for k in range(KW):
                    xs = xc[:, l0 + dilation * k : l0 + dilation * k + w]
                    nc.tensor.matmul(
                        zsig[:], wts_sb[:, WC_SIG + k * R : WC_SIG + (k + 1) * R], xs,
                        start=(k == 0), stop=False,
                    )
                nc.tensor.matmul(
                    zsig[:], wts_sb[0:CC, WCOND + R : WCOND + 2 * R], cc[:, l0 : l0 + w],
                    start=False, stop=True,
                )

                if prev is not None:
                    emit_outskip()

                th = hpool.tile([R, w], BF16, tag="th")
                nc.scalar.activation(th[:], ztan[:], AF.Tanh, bias=tan_bias)
                sg = hpool.tile([R, w], BF16, tag="sg")
                nc.scalar.activation(sg[:], zsig[:], AF.Sigmoid, bias=sig_bias)
                h = hpool.tile([R, w], BF16, tag="h")
                nc.vector.tensor_mul(h[:], th[:], sg[:])
                prev = (h, stg, gs, gw, l0)

            flush_ready()
        emit_outskip()
        flush_ready()

    nc.compile()
    return nc


def _pack_weights(w_conv, w_cond, w_out, w_skip):
    bf = ml_dtypes.bfloat16
    wts_p = np.zeros((CIN, WTS_COLS), dtype=bf)
    for k in range(KW):
        wts_p[:, WC_TAN + k * R : WC_TAN + (k + 1) * R] = w_conv[0:R, :, k].T.astype(bf)
        wts_p[:, WC_SIG + k * R : WC_SIG + (k + 1) * R] = (
            w_conv[R : 2 * R, :, k].T.astype(bf)
        )
    wts_p[0:CC, WCOND : WCOND + R] = w_cond[0:R, :, 0].T.astype(bf)
    wts_p[0:CC, WCOND + R : WCOND + 2 * R] = w_cond[R : 2 * R, :, 0].T.astype(bf)
    wts_p[:, WOS : WOS + R] = w_out[:, :, 0].T.astype(bf)
    wts_p[:, WOS + R : WOS + R + S] = w_skip[:, :, 0].T.astype(bf)
    return wts_p


def kernel(**inputs):
    x = np.asarray(inputs["x"], dtype=np.float32)
    cond = np.asarray(inputs["cond"], dtype=np.float32)
    w_conv = np.asarray(inputs["w_conv"], dtype=np.float32)
    b_conv = np.asarray(inputs["b_conv"], dtype=np.float32)
    w_cond = np.asarray(inputs["w_cond"], dtype=np.float32)
    b_cond = np.asarray(inputs["b_cond"], dtype=np.float32)
    w_out = np.asarray(inputs["w_out"], dtype=np.float32)
    b_out = np.asarray(inputs["b_out"], dtype=np.float32)
    w_skip = np.asarray(inputs["w_skip"], dtype=np.float32)
    b_skip = np.asarray(inputs["b_skip"], dtype=np.float32)
    dilation = int(np.asarray(inputs["dilation"]))
    pad = dilation * (KW - 1)

    zbias_p = np.stack(
        [b_conv[:R] + b_cond[:R], b_conv[R:] + b_cond[R:]], axis=1
    ).astype(np.float32)
    has_zbias = bool(zbias_p.any())

    key = (dilation, has_zbias)
    if key not in _built:
        _built[key] = _build(dilation, has_zbias)
    nc = _built[key]

    wts_p = _pack_weights(w_conv, w_cond, w_out, w_skip)
    bf = ml_dtypes.bfloat16
    xb = np.zeros((B, CIN, pad + T), dtype=bf)
    xb[:, :, pad:] = x.astype(bf)
    cb = np.ascontiguousarray(cond.astype(bf))

    in_maps = []
    for b in range(B):
        m = {"x": xb[b], "cond": cb[b], "wts": wts_p}
        if has_zbias:
            m["zbias"] = zbias_p
        in_maps.append(m)
    br = run_bass_kernel_spmd(nc, in_maps, list(range(N_CORES)), trace=_TRACE)
    global _last_results
    _last_results = br
    res = br.results
    output = np.empty((B, R, T), dtype=np.float32)
    skip = np.empty((B, S, T), dtype=np.float32)
    for b in range(B):
        ob = np.asarray(res[b]["outs"])
        for gs, gw in zip(CHUNK_STARTS, CHUNK_WIDTHS):
            output[b, :, gs : gs + gw] = ob[:, 2 * gs : 2 * gs + gw]
            skip[b, :, gs : gs + gw] = ob[:, 2 * gs + gw : 2 * gs + 2 * gw]
    if b_out.any():
        output = output + b_out[None, :, None]
    if b_skip.any():
        skip = skip + b_skip[None, :, None]
    return (output, skip)


# revision 19
# speedup vs baseline: 1.0499x; 1.0034x over previous
"""WaveNet-style gated dilated conv layer on 8 Trainium2 NeuronCores.

Strategy: data-parallel over batch (B=8 -> 1 batch element per core).
Per core (batch b):
  z_tanh = sum_k Wc_tanh[k] @ x[:, t-d*(2-k)] + Wcond_tanh @ cond + bias
  z_sig  = likewise for the second half of the 2R conv channels
  h      = tanh(z_tanh) * sigmoid(z_sig)
  out    = W_out @ h, skip = W_skip @ h  (1x1 convs)
All matmuls run in bf16 with fp32 PSUM accumulation.  x and cond are cast
to bf16 on host to halve HBM->SBUF traffic; x is also causal-padded on
host so no on-chip memset is needed.

HBM-traffic/trigger layout (all per core):
 - all weights are packed on host into ONE [128, 1280] bf16 tensor
   (6 conv-tap blocks | 2 cond blocks (rows 0:80) | out | skip), so the
   constant load is a single DMA trigger.
 - out and skip are written bf16 into ONE [128, 2T] DRAM tensor,
   chunk-interleaved ([out_chunk | skip_chunk] per chunk), so each chunk
   flush is a single DMA trigger and output traffic is halved vs fp32.
   Host de-interleaves and casts back to fp32.
 - each engine's dma_start goes to that engine's own DGE queue; one
   queue for everything is itself a bottleneck (~65-79us busy at the
   observed ~240 GB/s) and output flushes starve the input loads queued
   behind them near the tail.  Inputs (x, cond) ride the sync HWDGE
   queue; the weight load and output flushes ride the scalar HWDGE
   queue (gpsimd SWDGE measured slower and is avoided).

The out/skip 1x1 matmuls for tile i are issued during tile i+2's z
matmuls (two-tile software pipelining): h(i) comes out of a scalar
activation + vector multiply chain that trails the PE by most of a
tile, and the PE queue is FIFO past the ldweights window, so issuing
po/ps(i) right after z(i) stalls the PE ~0.4us per tile waiting on h.
Two tiles (~4.3us) of slack also absorb the ~0.6us holes the output
DMA triggers punch in the scalar activation stream at chunk ends.

TRN2 matmul instructions only have room for a single semaphore wait, so
the kernel is structured so no matmul ever needs two: input DMAs are
"observed" by the PE via standalone ldweights instructions before the
first matmul that would otherwise combine a DMA wait with a PSUM WAR
wait.
"""

import sys

for _p in ("/opt/trn_rl_repo",):
    if _p not in sys.path:
        sys.path.append(_p)

from contextlib import ExitStack

import ml_dtypes
import numpy as np

import concourse.bacc as bacc
import concourse.bass as bass
import concourse.tile as tile
from concourse import mybir
from concourse.bass_utils import run_bass_kernel_spmd

B, CIN, T = 8, 128, 16384
R, S, CC, KW = 128, 128, 80, 3
NT = 512           # time-tile width (one PSUM bank of fp32)
N_CORES = 8

BF16 = mybir.dt.bfloat16
FP32 = mybir.dt.float32
AF = mybir.ActivationFunctionType

# packed weight column offsets: 6 conv taps, 2 cond halves, out, skip
WC_TAN = 0              # 3 blocks of 128 (k=0,1,2), tanh half
WC_SIG = 3 * R          # 3 blocks of 128, sigmoid half
WCOND = 6 * R           # 2 blocks of 128 (rows 0:CC valid)
WOS = 8 * R             # out block then skip block
WTS_COLS = 10 * R

_built = {}
_TRACE = False        # set True (e.g. by a test harness) to capture an NTFF profile
_last_results = None  # BassKernelResults of the most recent run


# Streaming chunk widths: ramped at the head so each chunk's input DMA
# (~240 GB/s aggregate) lands before compute catches up to it, large in
# the middle (few DMA triggers), small at the tail (fast final drain).
CHUNK_WIDTHS = [512, 1024, 1536, 2048, 2560, 2560, 2560, 1536, 1024, 512, 512]
assert sum(CHUNK_WIDTHS) == T
CHUNK_STARTS = [sum(CHUNK_WIDTHS[:i]) for i in range(len(CHUNK_WIDTHS))]
NCH = len(CHUNK_WIDTHS)
PREFETCH = 2         # chunk lookahead beyond the current group
WARMUP_MM = 30       # narrow (N=128) cold matmuls covering the initial DMA
                     # latency: each retires in ~107ns so they bridge the
                     # ~3us until chunk-0 data + weights land without delaying
                     # the first real matmul, and keep the PE HAM window busy
                     # so the clock is at 8/8 when real work starts
TAIL_NT = 128        # tile width for the final chunk: the last tile's
                     # act+mul+out/skip+cast drain is exposed at the very end
                     # of the kernel, so make it 4x narrower there


def _build(dilation: int, has_zbias: bool) -> bass.Bass:
    pad = dilation * (KW - 1)

    nc = bacc.Bacc("TRN2", target_bir_lowering=False, debug=False, num_devices=N_CORES)

    x = nc.declare_dram_parameter("x", [CIN, pad + T], BF16, isOutput=False)
    cond = nc.declare_dram_parameter("cond", [CC, T], BF16, isOutput=False)
    wts = nc.declare_dram_parameter("wts", [CIN, WTS_COLS], BF16, isOutput=False)
    if has_zbias:
        zbias = nc.declare_dram_parameter("zbias", [R, 2], FP32, isOutput=False)

    outs = nc.declare_dram_parameter("outs", [R, 2 * T], BF16, isOutput=True)

    with tile.TileContext(nc) as tc, ExitStack() as ctx:
        consts = ctx.enter_context(tc.tile_pool(name="consts", bufs=1))
        inpool = ctx.enter_context(tc.tile_pool(name="inp", bufs=PREFETCH + 2))
        hpool = ctx.enter_context(tc.tile_pool(name="h", bufs=4))
        opool = ctx.enter_context(tc.tile_pool(name="o", bufs=3))
        zpsum = ctx.enter_context(tc.tile_pool(name="zpsum", bufs=2, space="PSUM"))
        opsum = ctx.enter_context(tc.tile_pool(name="opsum", bufs=2, space="PSUM"))

        # chunk 0 is loaded before anything else: every HWDGE trigger costs
        # ~0.6us of serial sequencer time, so the first-needed data goes first
        xc_tiles = [None] * NCH
        cc_tiles = [None] * NCH

        def load_chunk(g):
            gs, gw = CHUNK_STARTS[g], CHUNK_WIDTHS[g]
            xc = inpool.tile([CIN, pad + gw], BF16, tag="xc")
            nc.sync.dma_start(xc[:], x[:, gs : gs + pad + gw])
            cc = inpool.tile([CC, gw], BF16, tag="cc")
            nc.sync.dma_start(cc[:], cond[:, gs : gs + gw])
            xc_tiles[g], cc_tiles[g] = xc, cc

        wts_sb = consts.tile([CIN, WTS_COLS], BF16)
        nc.sync.dma_start(wts_sb[:], wts[:])
        load_chunk(0)
        if has_zbias:
            zbias_sb = consts.tile([R, 2], FP32)
            nc.scalar.dma_start(zbias_sb[:], zbias[:])
            tan_bias = zbias_sb[:, 0:1]
            sig_bias = zbias_sb[:, 1:2]
        else:
            tan_bias = 0.0
            sig_bias = 0.0
        load_chunk(1)

        # Warm-up during the input-load head: narrow matmuls on a zeroed
        # SBUF tile keep the PE HAM busy until real data arrives, and two
        # 1-column activations trigger the tanh/sigmoid table load (~2.7us).
        garbage = consts.tile([CIN, NT], BF16)
        act_sink = consts.tile([R, 1], FP32)
        nc.vector.memset(garbage[:], 0.0)
        nc.vector.memset(act_sink[:], 0.0)
        nc.scalar.activation(act_sink[:], act_sink[:], AF.Tanh, bias=tan_bias)
        nc.scalar.activation(act_sink[:], act_sink[:], AF.Sigmoid, bias=sig_bias)
        for _ in range(WARMUP_MM):
            wz = zpsum.tile([R, NT], FP32, tag="ztan")
            nc.tensor.matmul(
                wz[:, 0:R], garbage[:, 0:R], garbage[:, 0:R], start=True, stop=True
            )

        # pending = [(h, stg, gs, gw, l0, w), ...] tiles whose out/skip
        # matmuls haven't been issued yet (out/skip trail the z matmuls by
        # PIPE tiles); ready_flush = chunks whose staging tile is fully
        # written and can be DMAed out.
        PIPE = 2
        pending = []
        ready_flush = []

        def emit_outskip():
            ph, pstg, pgs, pgw, pl0, pw = pending.pop(0)
            po = opsum.tile([R, pw], FP32, tag="po")
            nc.tensor.matmul(
                po[:], wts_sb[:, WOS : WOS + R], ph[:], start=True, stop=True
            )
            ps = opsum.tile([S, pw], FP32, tag="ps")
            nc.tensor.matmul(
                ps[:], wts_sb[:, WOS + R : WOS + R + S], ph[:], start=True, stop=True
            )
            nc.vector.tensor_copy(pstg[:, pl0 : pl0 + pw], po[:])
            nc.vector.tensor_copy(pstg[:, pgw + pl0 : pgw + pl0 + pw], ps[:])
            if pl0 + pw == pgw:
                ready_flush.append((pgs, pgw, pstg))

        def flush_ready():
            while ready_flush:
                fgs, fgw, fstg = ready_flush.pop(0)
                nc.scalar.dma_start(outs[:, 2 * fgs : 2 * fgs + 2 * fgw], fstg[:])

        for g in range(NCH):
            gs, gw = CHUNK_STARTS[g], CHUNK_WIDTHS[g]
            for gg in range(g + 1, min(g + PREFETCH + 1, NCH)):
                if xc_tiles[gg] is None:
                    load_chunk(gg)
            xc, cc = xc_tiles[g], cc_tiles[g]
            # let PE observe the chunk DMA sems on standalone 1-column
            # ldweights (a full-width observer costs a ~430ns PE bubble at
            # every chunk boundary) so no accumulating matmul needs two waits
            nc.tensor.ldweights(xc[:, 0:1])
            nc.tensor.ldweights(cc[:, 0:1])

            nt = TAIL_NT if g == NCH - 1 else NT
            stg = opool.tile([R, 2 * gw], BF16, tag="stg")
            for l0 in range(0, gw, nt):
                w = min(nt, gw - l0)
                ztan = zpsum.tile([R, w], FP32, tag="ztan")
                zsig = zpsum.tile([R, w], FP32, tag="zsig")
                for k in range(KW):
                    xs = xc[:, l0 + dilation * k : l0 + dilation * k + w]
                    nc.tensor.matmul(
                        ztan[:], wts_sb[:, WC_TAN + k * R : WC_TAN + (k + 1) * R], xs,
                        start=(k == 0), stop=False,
                    )
                nc.tensor.matmul(
                    ztan[:], wts_sb[0:CC, WCOND : WCOND + R], cc[:, l0 : l0 + w],
                    start=False, stop=True,
                )
                for k in range(KW):
                    xs = xc[:, l0 + dilation * k : l0 + dilation * k + w]
                    nc.tensor.matmul(
                        zsig[:], wts_sb[:, WC_SIG + k * R : WC_SIG + (k + 1) * R], xs,
                        start=(k == 0), stop=False,
                    )
                nc.tensor.matmul(
                    zsig[:], wts_sb[0:CC, WCOND + R : WCOND + 2 * R], cc[:, l0 : l0 + w],
                    start=False, stop=True,
                )

                if len(pending) >= PIPE:
                    emit_outskip()

                th = hpool.tile([R, w], BF16, tag="th")
                nc.scalar.activation(th[:], ztan[:], AF.Tanh, bias=tan_bias)
                sg = hpool.tile([R, w], BF16, tag="sg")
                nc.scalar.activation(sg[:], zsig[:], AF.Sigmoid, bias=sig_bias)
                h = hpool.tile([R, w], BF16, tag="h")
                nc.vector.tensor_mul(h[:], th[:], sg[:])
                pending.append((h, stg, gs, gw, l0, w))

            flush_ready()
        while pending:
            emit_outskip()
        flush_ready()

    nc.compile()
    return nc


def _pack_weights(w_conv, w_cond, w_out, w_skip):
    bf = ml_dtypes.bfloat16
    wts_p = np.zeros((CIN, WTS_COLS), dtype=bf)
    for k in range(KW):
        wts_p[:, WC_TAN + k * R : WC_TAN + (k + 1) * R] = w_conv[0:R, :, k].T.astype(bf)
        wts_p[:, WC_SIG + k * R : WC_SIG + (k + 1) * R] = (
            w_conv[R : 2 * R, :, k].T.astype(bf)
        )
    wts_p[0:CC, WCOND : WCOND + R] = w_cond[0:R, :, 0].T.astype(bf)
    wts_p[0:CC, WCOND + R : WCOND + 2 * R] = w_cond[R : 2 * R, :, 0].T.astype(bf)
    wts_p[:, WOS : WOS + R] = w_out[:, :, 0].T.astype(bf)
    wts_p[:, WOS + R : WOS + R + S] = w_skip[:, :, 0].T.astype(bf)
    return wts_p


def kernel(**inputs):
    x = np.asarray(inputs["x"], dtype=np.float32)
    cond = np.asarray(inputs["cond"], dtype=np.float32)
    w_conv = np.asarray(inputs["w_conv"], dtype=np.float32)
    b_conv = np.asarray(inputs["b_conv"], dtype=np.float32)
    w_cond = np.asarray(inputs["w_cond"], dtype=np.float32)
    b_cond = np.asarray(inputs["b_cond"], dtype=np.float32)
    w_out = np.asarray(inputs["w_out"], dtype=np.float32)
    b_out = np.asarray(inputs["b_out"], dtype=np.float32)
    w_skip = np.asarray(inputs["w_skip"], dtype=np.float32)
    b_skip = np.asarray(inputs["b_skip"], dtype=np.float32)
    dilation = int(np.asarray(inputs["dilation"]))
    pad = dilation * (KW - 1)

    zbias_p = np.stack(
        [b_conv[:R] + b_cond[:R], b_conv[R:] + b_cond[R:]], axis=1
    ).astype(np.float32)
    has_zbias = bool(zbias_p.any())

    key = (dilation, has_zbias)
    if key not in _built:
        _built[key] = _build(dilation, has_zbias)
    nc = _built[key]

    wts_p = _pack_weights(w_conv, w_cond, w_out, w_skip)
    bf = ml_dtypes.bfloat16
    xb = np.zeros((B, CIN, pad + T), dtype=bf)
    xb[:, :, pad:] = x.astype(bf)
    cb = np.ascontiguousarray(cond.astype(bf))

    in_maps = []
    for b in range(B):
        m = {"x": xb[b], "cond": cb[b], "wts": wts_p}
        if has_zbias:
            m["zbias"] = zbias_p
        in_maps.append(m)
    br = run_bass_kernel_spmd(nc, in_maps, list(range(N_CORES)), trace=_TRACE)
    global _last_results
    _last_results = br
    res = br.results
    output = np.empty((B, R, T), dtype=np.float32)
    skip = np.empty((B, S, T), dtype=np.float32)
    for b in range(B):
        ob = np.asarray(res[b]["outs"])
        for gs, gw in zip(CHUNK_STARTS, CHUNK_WIDTHS):
            output[b, :, gs : gs + gw] = ob[:, 2 * gs : 2 * gs + gw]
            skip[b, :, gs : gs + gw] = ob[:, 2 * gs + gw : 2 * gs + 2 * gw]
    if b_out.any():
        output = output + b_out[None, :, None]
    if b_skip.any():
        skip = skip + b_skip[None, :, None]
    return (output, skip)


# revision 21
# speedup vs baseline: 1.0661x; 1.0154x over previous
"""WaveNet-style gated dilated conv layer on 8 Trainium2 NeuronCores.

Strategy: data-parallel over batch (B=8 -> 1 batch element per core).
Per core (batch b):
  z_tanh = sum_k Wc_tanh[k] @ x[:, t-d*(2-k)] + Wcond_tanh @ cond + bias
  z_sig  = likewise for the second half of the 2R conv channels
  h      = tanh(z_tanh) * sigmoid(z_sig)
  out    = W_out @ h, skip = W_skip @ h  (1x1 convs)
All matmuls run in bf16 with fp32 PSUM accumulation.  x and cond are cast
to bf16 on host to halve HBM->SBUF traffic; x is also causal-padded on
host so no on-chip memset is needed.

HBM-traffic/trigger layout (all per core):
 - all weights are packed on host into ONE [128, 1280] bf16 tensor
   (6 conv-tap blocks | 2 cond blocks (rows 0:80) | out | skip), so the
   constant load is a single DMA trigger.
 - out and skip are written bf16 into ONE [128, 2T] DRAM tensor,
   chunk-interleaved ([out_chunk | skip_chunk] per chunk), so each chunk
   flush is a single DMA trigger and output traffic is halved vs fp32.
   Host de-interleaves and casts back to fp32.
 - each engine's dma_start goes to that engine's own DGE queue; one
   queue for everything is itself a bottleneck (~65-79us busy at the
   observed ~240 GB/s) and output flushes starve the input loads queued
   behind them near the tail.  Inputs (x, cond) ride the sync HWDGE
   queue; the weight load and output flushes ride the scalar HWDGE
   queue (gpsimd SWDGE measured slower and is avoided).

The out/skip 1x1 matmuls for tile i are issued during tile i+2's z
matmuls (two-tile software pipelining): h(i) comes out of a scalar
activation + vector multiply chain that trails the PE by most of a
tile, and the PE queue is FIFO past the ldweights window, so issuing
po/ps(i) right after z(i) stalls the PE ~0.4us per tile waiting on h.
Two tiles (~4.3us) of slack also absorb the ~0.6us holes the output
DMA triggers punch in the scalar activation stream at chunk ends.

TRN2 matmul instructions only have room for a single semaphore wait, so
the kernel is structured so no matmul ever needs two: input DMAs are
"observed" by the PE via standalone ldweights instructions before the
first matmul that would otherwise combine a DMA wait with a PSUM WAR
wait.
"""

import sys

for _p in ("/opt/trn_rl_repo",):
    if _p not in sys.path:
        sys.path.append(_p)

from contextlib import ExitStack

import ml_dtypes
import numpy as np

import concourse.bacc as bacc
import concourse.bass as bass
import concourse.tile as tile
from concourse import mybir
from concourse.bass_utils import run_bass_kernel_spmd

B, CIN, T = 8, 128, 16384
R, S, CC, KW = 128, 128, 80, 3
NT = 512           # time-tile width (one PSUM bank of fp32)
N_CORES = 8

BF16 = mybir.dt.bfloat16
FP32 = mybir.dt.float32
AF = mybir.ActivationFunctionType

# packed weight column offsets: 6 conv taps, 2 cond halves, out, skip
WC_TAN = 0              # 3 blocks of 128 (k=0,1,2), tanh half
WC_SIG = 3 * R          # 3 blocks of 128, sigmoid half
WCOND = 6 * R           # 2 blocks of 128 (rows 0:CC valid)
WOS = 8 * R             # out block then skip block
WTS_COLS = 10 * R

_built = {}
_TRACE = False        # set True (e.g. by a test harness) to capture an NTFF profile
_last_results = None  # BassKernelResults of the most recent run


# Streaming chunk widths: ramped at the head so each chunk's input DMA
# (~240 GB/s aggregate) lands before compute catches up to it, large in
# the middle (few DMA triggers), small at the tail (fast final drain).
CHUNK_WIDTHS = [512, 1024, 1536, 2048, 2560, 2560, 2560, 1536, 1024, 512, 512]
assert sum(CHUNK_WIDTHS) == T
CHUNK_STARTS = [sum(CHUNK_WIDTHS[:i]) for i in range(len(CHUNK_WIDTHS))]
NCH = len(CHUNK_WIDTHS)
PREFETCH = 2         # chunk lookahead beyond the current group
WARMUP_MM = 20       # narrow (N=128) cold matmuls covering the initial DMA
                     # latency: each retires in ~107ns so they bridge the
                     # ~3us until chunk-0 data + weights land without delaying
                     # the first real matmul, and keep the PE HAM window busy
                     # so the clock is at 8/8 when real work starts
TAIL_NT = 128        # tile width for the final chunk: the last tile's
                     # act+mul+out/skip+cast drain is exposed at the very end
                     # of the kernel, so make it 4x narrower there


def _build(dilation: int, has_zbias: bool) -> bass.Bass:
    pad = dilation * (KW - 1)

    nc = bacc.Bacc("TRN2", target_bir_lowering=False, debug=False, num_devices=N_CORES)

    x = nc.declare_dram_parameter("x", [CIN, pad + T], BF16, isOutput=False)
    cond = nc.declare_dram_parameter("cond", [CC, T], BF16, isOutput=False)
    wts = nc.declare_dram_parameter("wts", [CIN, WTS_COLS], BF16, isOutput=False)
    if has_zbias:
        zbias = nc.declare_dram_parameter("zbias", [R, 2], FP32, isOutput=False)

    outs = nc.declare_dram_parameter("outs", [R, 2 * T], BF16, isOutput=True)

    with tile.TileContext(nc) as tc, ExitStack() as ctx:
        consts = ctx.enter_context(tc.tile_pool(name="consts", bufs=1))
        inpool = ctx.enter_context(tc.tile_pool(name="inp", bufs=PREFETCH + 2))
        hpool = ctx.enter_context(tc.tile_pool(name="h", bufs=4))
        opool = ctx.enter_context(tc.tile_pool(name="o", bufs=3))
        zpsum = ctx.enter_context(tc.tile_pool(name="zpsum", bufs=2, space="PSUM"))
        opsum = ctx.enter_context(tc.tile_pool(name="opsum", bufs=2, space="PSUM"))

        # chunk 0 is loaded before anything else: every HWDGE trigger costs
        # ~0.6us of serial sequencer time, so the first-needed data goes first
        xc_tiles = [None] * NCH
        cc_tiles = [None] * NCH

        def load_chunk(g):
            gs, gw = CHUNK_STARTS[g], CHUNK_WIDTHS[g]
            xc = inpool.tile([CIN, pad + gw], BF16, tag="xc")
            nc.sync.dma_start(xc[:], x[:, gs : gs + pad + gw])
            cc = inpool.tile([CC, gw], BF16, tag="cc")
            nc.sync.dma_start(cc[:], cond[:, gs : gs + gw])
            xc_tiles[g], cc_tiles[g] = xc, cc

        wts_sb = consts.tile([CIN, WTS_COLS], BF16)
        nc.sync.dma_start(wts_sb[:], wts[:])
        load_chunk(0)
        if has_zbias:
            zbias_sb = consts.tile([R, 2], FP32)
            nc.scalar.dma_start(zbias_sb[:], zbias[:])
            tan_bias = zbias_sb[:, 0:1]
            sig_bias = zbias_sb[:, 1:2]
        else:
            tan_bias = 0.0
            sig_bias = 0.0
        load_chunk(1)

        # Warm-up during the input-load head: narrow matmuls on a zeroed
        # SBUF tile keep the PE HAM busy until real data arrives, and two
        # 1-column activations trigger the tanh/sigmoid table load (~2.7us).
        garbage = consts.tile([CIN, NT], BF16)
        act_sink = consts.tile([R, 1], FP32)
        nc.vector.memset(garbage[:], 0.0)
        nc.vector.memset(act_sink[:], 0.0)
        nc.scalar.activation(act_sink[:], act_sink[:], AF.Tanh, bias=tan_bias)
        nc.scalar.activation(act_sink[:], act_sink[:], AF.Sigmoid, bias=sig_bias)
        for _ in range(WARMUP_MM):
            wz = zpsum.tile([R, NT], FP32, tag="ztan")
            nc.tensor.matmul(
                wz[:, 0:R], garbage[:, 0:R], garbage[:, 0:R], start=True, stop=True
            )

        # pending = [(h, stg, gs, gw, l0, w), ...] tiles whose out/skip
        # matmuls haven't been issued yet (out/skip trail the z matmuls by
        # PIPE tiles); ready_flush = chunks whose staging tile is fully
        # written and can be DMAed out.
        PIPE = 2
        pending = []
        ready_flush = []

        def emit_outskip():
            ph, pstg, pgs, pgw, pl0, pw = pending.pop(0)
            po = opsum.tile([R, pw], FP32, tag="po")
            nc.tensor.matmul(
                po[:], wts_sb[:, WOS : WOS + R], ph[:], start=True, stop=True
            )
            ps = opsum.tile([S, pw], FP32, tag="ps")
            nc.tensor.matmul(
                ps[:], wts_sb[:, WOS + R : WOS + R + S], ph[:], start=True, stop=True
            )
            nc.vector.tensor_copy(pstg[:, pl0 : pl0 + pw], po[:])
            nc.vector.tensor_copy(pstg[:, pgw + pl0 : pgw + pl0 + pw], ps[:])
            if pl0 + pw == pgw:
                ready_flush.append((pgs, pgw, pstg))

        def flush_ready():
            while ready_flush:
                fgs, fgw, fstg = ready_flush.pop(0)
                nc.scalar.dma_start(outs[:, 2 * fgs : 2 * fgs + 2 * fgw], fstg[:])

        for g in range(NCH):
            gs, gw = CHUNK_STARTS[g], CHUNK_WIDTHS[g]
            for gg in range(g + 1, min(g + PREFETCH + 1, NCH)):
                if xc_tiles[gg] is None:
                    load_chunk(gg)
            xc, cc = xc_tiles[g], cc_tiles[g]
            # let PE observe the chunk DMA sems on standalone 1-column
            # ldweights (a full-width observer costs a ~430ns PE bubble at
            # every chunk boundary) so no accumulating matmul needs two waits
            nc.tensor.ldweights(xc[:, 0:1])
            nc.tensor.ldweights(cc[:, 0:1])

            nt = TAIL_NT if g == NCH - 1 else NT
            stg = opool.tile([R, 2 * gw], BF16, tag="stg")
            for l0 in range(0, gw, nt):
                w = min(nt, gw - l0)
                ztan = zpsum.tile([R, w], FP32, tag="ztan")
                zsig = zpsum.tile([R, w], FP32, tag="zsig")
                for k in range(KW):
                    xs = xc[:, l0 + dilation * k : l0 + dilation * k + w]
                    nc.tensor.matmul(
                        ztan[:], wts_sb[:, WC_TAN + k * R : WC_TAN + (k + 1) * R], xs,
                        start=(k == 0), stop=False,
                    )
                nc.tensor.matmul(
                    ztan[:], wts_sb[0:CC, WCOND : WCOND + R], cc[:, l0 : l0 + w],
                    start=False, stop=True,
                )
                for k in range(KW):
                    xs = xc[:, l0 + dilation * k : l0 + dilation * k + w]
                    nc.tensor.matmul(
                        zsig[:], wts_sb[:, WC_SIG + k * R : WC_SIG + (k + 1) * R], xs,
                        start=(k == 0), stop=False,
                    )
                nc.tensor.matmul(
                    zsig[:], wts_sb[0:CC, WCOND + R : WCOND + 2 * R], cc[:, l0 : l0 + w],
                    start=False, stop=True,
                )

                if len(pending) >= PIPE:
                    emit_outskip()
                    flush_ready()

                th = hpool.tile([R, w], BF16, tag="th")
                nc.scalar.activation(th[:], ztan[:], AF.Tanh, bias=tan_bias)
                sg = hpool.tile([R, w], BF16, tag="sg")
                nc.scalar.activation(sg[:], zsig[:], AF.Sigmoid, bias=sig_bias)
                # the h multiply runs on the otherwise-idle gpsimd engine:
                # vector is nearly saturated by the two PSUM->SBUF casts per
                # tile (~0.7us each, PSUM reads are single-port on DVE)
                h = hpool.tile([R, w], BF16, tag="h")
                nc.gpsimd.tensor_mul(h[:], th[:], sg[:])
                pending.append((h, stg, gs, gw, l0, w))

        while pending:
            emit_outskip()
        flush_ready()

    nc.compile()
    return nc


def _pack_weights(w_conv, w_cond, w_out, w_skip):
    bf = ml_dtypes.bfloat16
    wts_p = np.zeros((CIN, WTS_COLS), dtype=bf)
    for k in range(KW):
        wts_p[:, WC_TAN + k * R : WC_TAN + (k + 1) * R] = w_conv[0:R, :, k].T.astype(bf)
        wts_p[:, WC_SIG + k * R : WC_SIG + (k + 1) * R] = (
            w_conv[R : 2 * R, :, k].T.astype(bf)
        )
    wts_p[0:CC, WCOND : WCOND + R] = w_cond[0:R, :, 0].T.astype(bf)
    wts_p[0:CC, WCOND + R : WCOND + 2 * R] = w_cond[R : 2 * R, :, 0].T.astype(bf)
    wts_p[:, WOS : WOS + R] = w_out[:, :, 0].T.astype(bf)
    wts_p[:, WOS + R : WOS + R + S] = w_skip[:, :, 0].T.astype(bf)
    return wts_p


def kernel(**inputs):
    x = np.asarray(inputs["x"], dtype=np.float32)
    cond = np.asarray(inputs["cond"], dtype=np.float32)
    w_conv = np.asarray(inputs["w_conv"], dtype=np.float32)
    b_conv = np.asarray(inputs["b_conv"], dtype=np.float32)
    w_cond = np.asarray(inputs["w_cond"], dtype=np.float32)
    b_cond = np.asarray(inputs["b_cond"], dtype=np.float32)
    w_out = np.asarray(inputs["w_out"], dtype=np.float32)
    b_out = np.asarray(inputs["b_out"], dtype=np.float32)
    w_skip = np.asarray(inputs["w_skip"], dtype=np.float32)
    b_skip = np.asarray(inputs["b_skip"], dtype=np.float32)
    dilation = int(np.asarray(inputs["dilation"]))
    pad = dilation * (KW - 1)

    zbias_p = np.stack(
        [b_conv[:R] + b_cond[:R], b_conv[R:] + b_cond[R:]], axis=1
    ).astype(np.float32)
    has_zbias = bool(zbias_p.any())

    key = (dilation, has_zbias)
    if key not in _built:
        _built[key] = _build(dilation, has_zbias)
    nc = _built[key]

    wts_p = _pack_weights(w_conv, w_cond, w_out, w_skip)
    bf = ml_dtypes.bfloat16
    xb = np.zeros((B, CIN, pad + T), dtype=bf)
    xb[:, :, pad:] = x.astype(bf)
    cb = np.ascontiguousarray(cond.astype(bf))

    in_maps = []
    for b in range(B):
        m = {"x": xb[b], "cond": cb[b], "wts": wts_p}
        if has_zbias:
            m["zbias"] = zbias_p
        in_maps.append(m)
    br = run_bass_kernel_spmd(nc, in_maps, list(range(N_CORES)), trace=_TRACE)
    global _last_results
    _last_results = br
    res = br.results
    output = np.empty((B, R, T), dtype=np.float32)
    skip = np.empty((B, S, T), dtype=np.float32)
    for b in range(B):
        ob = np.asarray(res[b]["outs"])
        for gs, gw in zip(CHUNK_STARTS, CHUNK_WIDTHS):
            output[b, :, gs : gs + gw] = ob[:, 2 * gs : 2 * gs + gw]
            skip[b, :, gs : gs + gw] = ob[:, 2 * gs + gw : 2 * gs + 2 * gw]
    if b_out.any():
        output = output + b_out[None, :, None]
    if b_skip.any():
        skip = skip + b_skip[None, :, None]
    return (output, skip)
